# revision 20
# baseline (speedup 1.0000x reference)
"""Trainium2 Bass kernel for nn_AttentionBlock (causal bilinear attention).

Reference computation (N_NET=1, D=4, N_H=8, N_T=2048, N_IN=N_OUT=256):
    Omega[N,b,h,t,u] = r'[N,b,t,i] Q[N,h,i,j] r'[N,b,u,j]
    Omega *= tril(ones(T, T))                      # causal mask (u <= t)
    r[N,b,t,i] = Omega[N,b,h,t,u] E[N,h,i,j] r'[N,b,u,j]   # sums over h

There is no softmax, so this is exact causal LINEAR attention and the
chunked prefix-state algorithm applies. With A_h = r' Q_h  [t, j] and
V_h[u] = E_h r'_u  [u, i], for chunk k of size C=128:

    out[t in k]  = sum_h A_h[t] @ P_h(k)          # inter-chunk (prefix state)
                 + sum_h sum_{u in k, u<=t} Omega[t,u] V_h[u]   # intra-chunk
    P_h(k+1)     = P_h(k) + r'[k-chunk]^T @ V_h[k-chunk]        # [j, i] state

This computes ~164k PE columns per core instead of ~344k for the direct
block-causal algorithm (the T x T score matrix never materializes beyond
one 128x128 diagonal block per head).

Sharding across 8 NeuronCores: core c handles batch b = c//2 and the 4
heads [4*(c%2), 4*(c%2)+4). Each core produces the partial output for its
batch summed over its 4 heads; the host adds the two head-group partials.

Per-core device algorithm (all matmuls fp16 with fp32 PSUM accumulate):
  Phase A:  A_sb[h][jc](j, t) = sum_i Q_h(i, j-blk) rT(i, t)   [j on parts]
  Chunk loop k = 0..15 (chunk = 128 positions):
    V(u,i)     = sum_j rT(j, u-chunk) ET_h(j, i)     (2 heads per matmul)
    S(u,t)     = sum_j rT(j, u-chunk) A_sb[h][jc](j, t-chunk)  = Omega^T
    s_sb       = S * tri_mask  (keep u <= t; DVE, fp16 out)
    apply:  po(t,i) += A_sb[h][jb](j, t-chunk)^T @ P_sb[h](j-blk, i)
    state:  P_ps[h](j,i) += rN(u-chunk, j-blk)^T @ V   (PSUM-resident prefix)
    intra:  po(t,i) += s_sb(u, t)^T @ V(u, i)
    P_sb[h] is the fp16 copy of P_ps[h] taken BEFORE this chunk's state
    update (so it holds the prefix over chunks < k).
Output is produced in natural [t, i] layout (no host transpose).
"""

import numpy as np

N_T = 2048
N_IN = 256
CH = 128            # chunk size
NCH = N_T // CH     # 16 chunks
T_TILE = 512
TT = N_T // T_TILE  # 4 t-tiles for phase A
HL = 4              # heads per core
N_CORES = 8

FP8_INTRA = True  # intra-chunk O matmuls in fp8e4m3 DoubleRow (2 rows/cycle);
                  # intra is ~1/4 of output rms, so the ~3.5% fp8 path error
                  # adds ~0.8% overall — well under the 2e-2 gate.

_cache = {}


def _tri_mask():
    # mask[u, t] = 1 if t >= u  (keep u <= t on the diagonal block)
    idx = np.arange(128)
    return (idx[None, :] >= idx[:, None]).astype(np.float32)


def _build_nc(repeat=1, bf16=False):
    import concourse.tile as tile
    import concourse.mybir as mybir
    from concourse import bacc

    F32 = mybir.dt.float32
    F16 = {False: mybir.dt.float32r, True: mybir.dt.bfloat16,
           "fp16": mybir.dt.float16}[bf16]
    F8 = mybir.dt.float8e4

    nc = bacc.Bacc("TRN2", target_bir_lowering=False, debug=False,
                   num_devices=N_CORES)
    rT_d = nc.dram_tensor("rT", (2, 128, N_T), F16, kind="ExternalInput").ap()
    rN_d = nc.dram_tensor("rN", (128, NCH, N_IN), F16,
                          kind="ExternalInput").ap()
    Q4_d = nc.dram_tensor("Q4", (HL, 2, 128, N_IN), F16,
                          kind="ExternalInput").ap()
    ET4_d = nc.dram_tensor("ET4", (HL, 2, 128, N_IN), F16,
                           kind="ExternalInput").ap()
    mask_d = nc.dram_tensor("mask", (128, 128), F16,
                            kind="ExternalInput").ap()
    out_d = nc.dram_tensor("out", (N_T, N_IN), F32,
                           kind="ExternalOutput").ap()

    # running per-engine cost estimates for greedy DVE/ACT balancing
    eng_load = {"v": 0.0, "s": 0.0}

    def copy_psum(out_ap, in_ap, n):
        # calibrated to TimelineSim engine-busy: both engines land near
        # ~600ns for a 512-col PSUM copy
        dve = n / 0.96 + 120.0
        act = n / 1.2 + 180.0
        if eng_load["v"] + dve <= eng_load["s"] + act:
            eng_load["v"] += dve
            nc.vector.tensor_copy(out_ap, in_ap)
        else:
            eng_load["s"] += act
            nc.scalar.copy(out_ap, in_ap)

    with tile.TileContext(nc) as tc:
        with (
            tc.tile_pool(name="const", bufs=1) as const,
            tc.tile_pool(name="vpool", bufs=8) as vpool,
            tc.tile_pool(name="spool", bufs=8) as spool,
            tc.tile_pool(name="ppool", bufs=8) as ppool,
            tc.tile_pool(name="opool", bufs=3) as opool,
            tc.tile_pool(name="psum", bufs=3, space="PSUM") as psum,
            tc.tile_pool(name="pstate", bufs=4, space="PSUM") as pstate,
            tc.tile_pool(name="psout", bufs=1, space="PSUM") as psout,
        ):
            # --- PE warm-up: dummy matmuls on memset data run during the
            # input-DMA lead-in so the HAM un-throttles (1.2->2.4 GHz) ---
            warm_f32 = const.tile([128, 128], F32)
            nc.vector.memset(warm_f32, 0.0)
            warm_sb = const.tile([128, 128], F16)
            nc.vector.tensor_copy(warm_sb, warm_f32)
            warm_ps = psum.tile([128, T_TILE], F32, tag="ps", name="warm_ps")
            for _w in range(24):
                nc.tensor.matmul(warm_ps[:, :128], warm_sb, warm_sb,
                                 start=True, stop=True, skip_group_check=True)

            # --- inputs ---
            mask_sb = const.tile([128, 128], F16)
            mask4 = const.tile([128, HL, CH], F16, name="mask4")
            Q_h = [const.tile([128, 2, N_IN], F16, name=f"Qh{h}")
                   for h in range(HL)]
            rT_t = [[const.tile([128, T_TILE], F16, name=f"rT{ic}_{tq}")
                     for tq in range(TT)] for ic in range(2)]
            rN_sb = const.tile([128, NCH, N_IN], F16, name="rN")
            ET_p = [const.tile([128, 2, 2, N_IN], F16, name=f"ETp{p}")
                    for p in range(2)]
            for ic in range(2):
                nc.sync.dma_start(out=Q_h[0][:, ic, :], in_=Q4_d[0, ic])
            for tq in range(TT):
                for ic in range(2):
                    nc.sync.dma_start(
                        out=rT_t[ic][tq],
                        in_=rT_d[ic, :, T_TILE * tq:T_TILE * (tq + 1)])
            for hl in range(1, HL):
                for ic in range(2):
                    nc.sync.dma_start(out=Q_h[hl][:, ic, :], in_=Q4_d[hl, ic])
            nc.sync.dma_start(out=rN_sb, in_=rN_d)
            for p2 in range(2):
                for jc in range(2):
                    for h2 in range(2):
                        nc.sync.dma_start(out=ET_p[p2][:, jc, h2, :],
                                          in_=ET4_d[2 * p2 + h2, jc])
            nc.sync.dma_start(out=mask_sb, in_=mask_d)
            for hl in range(HL):
                nc.sync.dma_start(out=mask4[:, hl, :], in_=mask_d)

            # chunk k of rT lives in tile [k // 4], columns 128*(k % 4)
            def rT_ch(jc, k):
                c0 = 128 * (k % 4)
                return rT_t[jc][k // 4][:, c0:c0 + 128]

            A4_sb = [const.tile([128, HL, N_T], F16, name=f"A4_{j}")
                     for j in range(2)]

            def body():
                # ---- Phase A: A_sb[h][jc](j, t) for all t ----
                # tq-pairs with ic outer so each Q stationary serves two
                # 512-col matmuls before reloading (half the weight loads).
                for hl in range(HL):
                    for jc in range(2):
                        for tp in range(TT // 2):
                            ps_p = [psum.tile([128, T_TILE], F32, tag="ps",
                                              name="ps_a") for _ in range(2)]
                            for ic in range(2):
                                for tq2 in range(2):
                                    nc.tensor.matmul(
                                        ps_p[tq2],
                                        Q_h[hl][:, ic,
                                                128 * jc:128 * (jc + 1)],
                                        rT_t[ic][2 * tp + tq2],
                                        start=(ic == 0), stop=(ic == 1))
                            for tq2 in range(2):
                                tq = 2 * tp + tq2
                                copy_psum(
                                    A4_sb[jc][:, hl,
                                              T_TILE * tq:T_TILE * (tq + 1)],
                                    ps_p[tq2], T_TILE)

                # ---- persistent state PSUM, keyed [p2][jb]; each bank holds
                # P[j in jb-block, h2, i] for the two heads of pair p2 ----
                P_ps = [[pstate.tile([128, 2, N_IN], F32, tag="pp",
                                     name=f"P{p2}_{jb}") for jb in range(2)]
                        for p2 in range(2)]

                # V for chunk k is computed during chunk k-1 (one-chunk
                # prefetch) so the psum->sbuf->fp8 copy chain is off the
                # critical path of the consuming chunk.
                def emit_v(k):
                    ps_v = [psum.tile([128, 2, N_IN], F32, tag="ps",
                                      name=f"ps_v{p2}") for p2 in range(2)]
                    for jc in range(2):
                        w = rT_ch(jc, k)
                        for p2 in range(2):
                            nc.tensor.matmul(ps_v[p2], w,
                                             ET_p[p2][:, jc, :, :],
                                             start=(jc == 0), stop=(jc == 1))
                    vt, vt8 = [], []
                    for p2 in range(2):
                        v_sb = vpool.tile([128, 2, N_IN], F16, tag="v",
                                          name="v_sb")
                        copy_psum(v_sb, ps_v[p2], 2 * N_IN)
                        vt.append(v_sb)
                        if FP8_INTRA:
                            v8 = vpool.tile([128, 2, N_IN], F8, tag="v8",
                                            name="v8_sb")
                            copy_psum(v8, ps_v[p2], 2 * N_IN)
                            vt8.append(v8)
                    return vt, vt8

                v_next = emit_v(0)

                # ---- chunk loop ----
                for k in range(NCH):
                    t0 = CH * k
                    # S scores (4 heads, one bank); jc-outer stationary.
                    # NOTE: PSUM 'start' zeroes the whole 2KB bank, so only
                    # the first matmul into each bank may set it; later
                    # sub-groups accumulate into the zeroed region.
                    ps_s = psum.tile([128, HL, CH], F32, tag="ps",
                                     name="ps_s")
                    for jc in range(2):
                        nc.tensor.matmul(
                            ps_s, rT_ch(jc, k), A4_sb[jc][:, :, t0:t0 + CH],
                            start=(jc == 0), stop=(jc == 1),
                            skip_group_check=True)
                    vt, vt8 = v_next
                    if k + 1 < NCH:
                        v_next = emit_v(k + 1)
                    # prefix-state copies for the apply step (chunks < k)
                    P_sb = None
                    if k >= 1:
                        P_sb = [[ppool.tile([128, 2, N_IN], F16, tag="p",
                                            name=f"P_sb{p2}_{jb}")
                                 for jb in range(2)] for p2 in range(2)]
                        for p2 in range(2):
                            for jb in range(2):
                                copy_psum(P_sb[p2][jb], P_ps[p2][jb],
                                          2 * N_IN)
                    # masked scores (all 4 heads in one DVE op); written
                    # directly in fp8 when the intra path runs in fp8
                    s4 = spool.tile([128, HL, CH], F8 if FP8_INTRA else F16,
                                    tag="s", name="s_sb")
                    nc.vector.tensor_mul(s4, ps_s, mask4)
                    eng_load["v"] += HL * CH / 0.96 + 150.0
                    s_sb = [s4[:, hl, :] for hl in range(HL)]

                    if k % 2 == 0:
                        po_pair = psout.tile([128, 2, N_IN], F32, tag="po",
                                             name="po")
                    po = po_pair[:, k % 2, :]
                    # 'start' zeroes the whole pair bank: only the first
                    # matmul of the EVEN chunk sets it.
                    bank_start = (k % 2 == 0)
                    # inter-chunk apply: po(t,i) += A^T P  (8 matmuls)
                    if k >= 1:
                        for hl in range(HL):
                            for jb in range(2):
                                nc.tensor.matmul(
                                    po, A4_sb[jb][:, hl, t0:t0 + CH],
                                    P_sb[hl // 2][jb][:, hl % 2, :],
                                    start=(bank_start and hl == 0
                                           and jb == 0),
                                    stop=False, skip_group_check=True)
                    # state update: P[p2][jb] += rN^T V, both heads per
                    # matmul; jb-outer so one rN stationary serves 2 matmuls
                    if k < NCH - 1:
                        for jb in range(2):
                            for p2 in range(2):
                                nc.tensor.matmul(
                                    P_ps[p2][jb],
                                    rN_sb[:, k, 128 * jb:128 * (jb + 1)],
                                    vt[p2],
                                    start=(k == 0), stop=(k == NCH - 2),
                                    skip_group_check=True)
                    # intra-chunk: po(t,i) += s^T V; with FP8_INTRA one
                    # DoubleRow matmul per head-pair sums both heads
                    if FP8_INTRA:
                        for p2 in range(2):
                            nc.tensor.matmul(
                                po, s4[:, 2 * p2:2 * p2 + 2, :], vt8[p2],
                                perf_mode=mybir.MatmulPerfMode.DoubleRow,
                                start=(k == 0 and p2 == 0),
                                stop=(k % 2 == 1 and p2 == 1),
                                skip_group_check=True)
                    else:
                        for hl in range(HL):
                            nc.tensor.matmul(
                                po, s_sb[hl], vt[hl // 2][:, hl % 2, :],
                                start=(k == 0 and hl == 0),
                                stop=(k % 2 == 1 and hl == HL - 1),
                                skip_group_check=True)
                    # drain the pair's output every other chunk
                    if k % 2 == 1:
                        ot = opool.tile([128, 2, N_IN], F32, tag="ot",
                                        name="ot")
                        copy_psum(ot, po_pair, 2 * N_IN)
                        for s in range(2):
                            nc.sync.dma_start(
                                out=out_d[t0 - CH + s * CH:
                                          t0 + s * CH, :],
                                in_=ot[:, s, :])

            if repeat == 1:
                body()
            elif repeat < 0:  # unrolled repeat (timing experiments)
                for _ in range(-repeat):
                    body()
            else:
                with tc.For_i(0, repeat, 1):
                    body()
    nc.compile()
    return nc


def _prep_in_maps(r_prime, E, Q, bf16=False):
    if bf16 == "fp16":
        cast_dt = np.float16
    elif bf16:
        import ml_dtypes
        cast_dt = ml_dtypes.bfloat16
    else:
        cast_dt = np.float32
    mask = _tri_mask()
    in_maps = []
    for c in range(N_CORES):
        b, hg = divmod(c, 2)
        heads = slice(4 * hg, 4 * hg + 4)
        rb = r_prime[0, b]                       # [T, I]
        rT = np.ascontiguousarray(rb.T).reshape(2, 128, N_T)
        rN = np.ascontiguousarray(
            rb.reshape(NCH, 128, N_IN).transpose(1, 0, 2))  # [u%128, k, j]
        Q4 = np.ascontiguousarray(Q[0, heads]).reshape(HL, 2, 128, N_IN)
        ET4 = np.ascontiguousarray(
            E[0, heads].transpose(0, 2, 1)).reshape(HL, 2, 128, N_IN)
        in_maps.append({"rT": rT.astype(cast_dt),
                        "rN": rN.astype(cast_dt),
                        "Q4": Q4.astype(cast_dt),
                        "ET4": ET4.astype(cast_dt),
                        "mask": mask.astype(cast_dt)})
    return in_maps


DTYPE = "fp16"  # float16 matmuls: full PE rate + fast weight loads


def kernel(r_prime, E, Q):
    from concourse import bass_utils

    if "nc" not in _cache:
        _cache["nc"] = _build_nc(bf16=DTYPE)
    nc = _cache["nc"]
    in_maps = _prep_in_maps(r_prime, E, Q, bf16=DTYPE)
    res = bass_utils.run_bass_kernel_spmd(nc, in_maps,
                                          core_ids=list(range(N_CORES)))
    out = np.zeros((1, 4, N_T, N_IN), dtype=np.float32)
    for b in range(4):
        out[0, b] = res.results[2 * b]["out"] + res.results[2 * b + 1]["out"]
    return out


# revision 23
# speedup vs baseline: 1.0121x; 1.0121x over previous
"""Trainium2 Bass kernel for nn_AttentionBlock (causal bilinear attention).

Reference computation (N_NET=1, D=4, N_H=8, N_T=2048, N_IN=N_OUT=256):
    Omega[N,b,h,t,u] = r'[N,b,t,i] Q[N,h,i,j] r'[N,b,u,j]
    Omega *= tril(ones(T, T))                      # causal mask (u <= t)
    r[N,b,t,i] = Omega[N,b,h,t,u] E[N,h,i,j] r'[N,b,u,j]   # sums over h

There is no softmax, so this is exact causal LINEAR attention and the
chunked prefix-state algorithm applies. With A_h = r' Q_h  [t, j] and
V_h[u] = E_h r'_u  [u, i], for chunk k of size C=128:

    out[t in k]  = sum_h A_h[t] @ P_h(k)          # inter-chunk (prefix state)
                 + sum_h sum_{u in k, u<=t} Omega[t,u] V_h[u]   # intra-chunk
    P_h(k+1)     = P_h(k) + r'[k-chunk]^T @ V_h[k-chunk]        # [j, i] state

This computes ~164k PE columns per core instead of ~344k for the direct
block-causal algorithm (the T x T score matrix never materializes beyond
one 128x128 diagonal block per head).

Sharding across 8 NeuronCores: core c handles batch b = c//2 and the 4
heads [4*(c%2), 4*(c%2)+4). Each core produces the partial output for its
batch summed over its 4 heads; the host adds the two head-group partials.

Per-core device algorithm (all matmuls fp16 with fp32 PSUM accumulate):
  Phase A:  A_sb[h][jc](j, t) = sum_i Q_h(i, j-blk) rT(i, t)   [j on parts]
  Chunk loop k = 0..15 (chunk = 128 positions):
    V(u,i)     = sum_j rT(j, u-chunk) ET_h(j, i)     (2 heads per matmul)
    S(u,t)     = sum_j rT(j, u-chunk) A_sb[h][jc](j, t-chunk)  = Omega^T
    s_sb       = S * tri_mask  (keep u <= t; DVE, fp16 out)
    apply:  po(t,i) += A_sb[h][jb](j, t-chunk)^T @ P_sb[h](j-blk, i)
    state:  P_ps[h](j,i) += rN(u-chunk, j-blk)^T @ V   (PSUM-resident prefix)
    intra:  po(t,i) += s_sb(u, t)^T @ V(u, i)
    P_sb[h] is the fp16 copy of P_ps[h] taken BEFORE this chunk's state
    update (so it holds the prefix over chunks < k).
Output is produced in natural [t, i] layout (no host transpose).
"""

import numpy as np

N_T = 2048
N_IN = 256
CH = 128            # chunk size
NCH = N_T // CH     # 16 chunks
T_TILE = 512
TT = N_T // T_TILE  # 4 t-tiles for phase A
HL = 4              # heads per core
N_CORES = 8

FP8_INTRA = True  # intra-chunk O matmuls in fp8e4m3 DoubleRow (2 rows/cycle);
                  # intra is ~1/4 of output rms, so the ~3.5% fp8 path error
                  # adds ~0.8% overall — well under the 2e-2 gate.

_cache = {}


def _tri_mask():
    # mask[u, t] = 1 if t >= u  (keep u <= t on the diagonal block)
    idx = np.arange(128)
    return (idx[None, :] >= idx[:, None]).astype(np.float32)


def _build_nc(repeat=1, bf16=False):
    import concourse.tile as tile
    import concourse.mybir as mybir
    from concourse import bacc

    F32 = mybir.dt.float32
    F16 = {False: mybir.dt.float32r, True: mybir.dt.bfloat16,
           "fp16": mybir.dt.float16}[bf16]
    F8 = mybir.dt.float8e4

    nc = bacc.Bacc("TRN2", target_bir_lowering=False, debug=False,
                   num_devices=N_CORES)
    rT_d = nc.dram_tensor("rT", (2, 128, N_T), F16, kind="ExternalInput").ap()
    rN_d = nc.dram_tensor("rN", (128, NCH, N_IN), F16,
                          kind="ExternalInput").ap()
    Q4_d = nc.dram_tensor("Q4", (HL, 2, 128, N_IN), F16,
                          kind="ExternalInput").ap()
    ET4_d = nc.dram_tensor("ET4", (HL, 2, 128, N_IN), F16,
                           kind="ExternalInput").ap()
    mask_d = nc.dram_tensor("mask", (128, 128), F16,
                            kind="ExternalInput").ap()
    out_d = nc.dram_tensor("out", (N_T, N_IN), F32,
                           kind="ExternalOutput").ap()

    # running per-engine cost estimates for greedy DVE/ACT balancing
    eng_load = {"v": 0.0, "s": 0.0}

    def copy_psum(out_ap, in_ap, n):
        # calibrated to TimelineSim engine-busy: both engines land near
        # ~600ns for a 512-col PSUM copy
        dve = n / 0.96 + 120.0
        act = n / 1.2 + 180.0
        if eng_load["v"] + dve <= eng_load["s"] + act:
            eng_load["v"] += dve
            nc.vector.tensor_copy(out_ap, in_ap)
        else:
            eng_load["s"] += act
            nc.scalar.copy(out_ap, in_ap)

    with tile.TileContext(nc) as tc:
        with (
            tc.tile_pool(name="const", bufs=1) as const,
            tc.tile_pool(name="vpool", bufs=8) as vpool,
            tc.tile_pool(name="spool", bufs=8) as spool,
            tc.tile_pool(name="ppool", bufs=8) as ppool,
            tc.tile_pool(name="opool", bufs=3) as opool,
            tc.tile_pool(name="psum", bufs=3, space="PSUM") as psum,
            tc.tile_pool(name="pstate", bufs=4, space="PSUM") as pstate,
            tc.tile_pool(name="psout", bufs=1, space="PSUM") as psout,
        ):
            # --- PE warm-up: dummy matmuls on memset data run during the
            # input-DMA lead-in so the HAM un-throttles (1.2->2.4 GHz) ---
            warm_f32 = const.tile([128, 128], F32)
            nc.vector.memset(warm_f32, 0.0)
            warm_sb = const.tile([128, 128], F16)
            nc.vector.tensor_copy(warm_sb, warm_f32)
            warm_ps = psum.tile([128, T_TILE], F32, tag="ps", name="warm_ps")
            for _w in range(24):
                nc.tensor.matmul(warm_ps[:, :128], warm_sb, warm_sb,
                                 start=True, stop=True, skip_group_check=True)

            # --- inputs ---
            mask_sb = const.tile([128, 128], F16)
            mask4 = const.tile([128, HL, CH], F16, name="mask4")
            Q_h = [const.tile([128, 2, N_IN], F16, name=f"Qh{h}")
                   for h in range(HL)]
            rT_t = [[const.tile([128, T_TILE], F16, name=f"rT{ic}_{tq}")
                     for tq in range(TT)] for ic in range(2)]
            rN_sb = const.tile([128, NCH, N_IN], F16, name="rN")
            ET_p = [const.tile([128, 2, 2, N_IN], F16, name=f"ETp{p}")
                    for p in range(2)]
            for ic in range(2):
                nc.sync.dma_start(out=Q_h[0][:, ic, :], in_=Q4_d[0, ic])
            for tq in range(TT):
                for ic in range(2):
                    nc.sync.dma_start(
                        out=rT_t[ic][tq],
                        in_=rT_d[ic, :, T_TILE * tq:T_TILE * (tq + 1)])
            for hl in range(1, HL):
                for ic in range(2):
                    nc.sync.dma_start(out=Q_h[hl][:, ic, :], in_=Q4_d[hl, ic])
            nc.sync.dma_start(out=rN_sb, in_=rN_d)
            for p2 in range(2):
                for jc in range(2):
                    for h2 in range(2):
                        nc.sync.dma_start(out=ET_p[p2][:, jc, h2, :],
                                          in_=ET4_d[2 * p2 + h2, jc])
            nc.sync.dma_start(out=mask_sb, in_=mask_d)
            for hl in range(HL):
                nc.sync.dma_start(out=mask4[:, hl, :], in_=mask_d)

            # chunk k of rT lives in tile [k // 4], columns 128*(k % 4)
            def rT_ch(jc, k):
                c0 = 128 * (k % 4)
                return rT_t[jc][k // 4][:, c0:c0 + 128]

            A4_sb = [const.tile([128, HL, N_T], F16, name=f"A4_{j}")
                     for j in range(2)]

            def body():
                # ---- Phase A: A_sb[h][jc](j, t) for all t ----
                # tq-pairs with ic outer so each Q stationary serves two
                # 512-col matmuls before reloading (half the weight loads).
                for hl in range(HL):
                    for jc in range(2):
                        for tp in range(TT // 2):
                            ps_p = [psum.tile([128, T_TILE], F32, tag="ps",
                                              name="ps_a") for _ in range(2)]
                            for ic in range(2):
                                for tq2 in range(2):
                                    nc.tensor.matmul(
                                        ps_p[tq2],
                                        Q_h[hl][:, ic,
                                                128 * jc:128 * (jc + 1)],
                                        rT_t[ic][2 * tp + tq2],
                                        start=(ic == 0), stop=(ic == 1))
                            for tq2 in range(2):
                                tq = 2 * tp + tq2
                                copy_psum(
                                    A4_sb[jc][:, hl,
                                              T_TILE * tq:T_TILE * (tq + 1)],
                                    ps_p[tq2], T_TILE)

                # ---- persistent state PSUM, keyed [p2][jb]; each bank holds
                # P[j in jb-block, h2, i] for the two heads of pair p2 ----
                P_ps = [[pstate.tile([128, 2, N_IN], F32, tag="pp",
                                     name=f"P{p2}_{jb}") for jb in range(2)]
                        for p2 in range(2)]

                # V for chunk k is computed during chunk k-1 (one-chunk
                # prefetch) so the psum->sbuf->fp8 copy chain is off the
                # critical path of the consuming chunk.
                def emit_v(k):
                    ps_v = [psum.tile([128, 2, N_IN], F32, tag="ps",
                                      name=f"ps_v{p2}") for p2 in range(2)]
                    for jc in range(2):
                        w = rT_ch(jc, k)
                        for p2 in range(2):
                            nc.tensor.matmul(ps_v[p2], w,
                                             ET_p[p2][:, jc, :, :],
                                             start=(jc == 0), stop=(jc == 1))
                    vt, vt8 = [], []
                    for p2 in range(2):
                        v_sb = vpool.tile([128, 2, N_IN], F16, tag="v",
                                          name="v_sb")
                        copy_psum(v_sb, ps_v[p2], 2 * N_IN)
                        vt.append(v_sb)
                        if FP8_INTRA:
                            v8 = vpool.tile([128, 2, N_IN], F8, tag="v8",
                                            name="v8_sb")
                            copy_psum(v8, ps_v[p2], 2 * N_IN)
                            vt8.append(v8)
                    return vt, vt8

                v_next = emit_v(0)

                # ---- chunk loop ----
                for k in range(NCH):
                    t0 = CH * k
                    # S scores (4 heads, one bank); jc-outer stationary.
                    # NOTE: PSUM 'start' zeroes the whole 2KB bank, so only
                    # the first matmul into each bank may set it; later
                    # sub-groups accumulate into the zeroed region.
                    ps_s = psum.tile([128, HL, CH], F32, tag="ps",
                                     name="ps_s")
                    for jc in range(2):
                        nc.tensor.matmul(
                            ps_s, rT_ch(jc, k), A4_sb[jc][:, :, t0:t0 + CH],
                            start=(jc == 0), stop=(jc == 1),
                            skip_group_check=True)
                    vt, vt8 = v_next
                    # prefix-state copies for the apply step (chunks < k);
                    # emitted BEFORE the V(k+1) copies so they sit ahead of
                    # them in the in-order engine queues (their dependency,
                    # state(k-1), is already satisfied at chunk entry)
                    P_sb = None
                    if k >= 1:
                        P_sb = [[ppool.tile([128, 2, N_IN], F16, tag="p",
                                            name=f"P_sb{p2}_{jb}")
                                 for jb in range(2)] for p2 in range(2)]
                        for p2 in range(2):
                            for jb in range(2):
                                copy_psum(P_sb[p2][jb], P_ps[p2][jb],
                                          2 * N_IN)
                    if k + 1 < NCH:
                        v_next = emit_v(k + 1)
                    # masked scores (all 4 heads in one DVE op); written
                    # directly in fp8 when the intra path runs in fp8
                    s4 = spool.tile([128, HL, CH], F8 if FP8_INTRA else F16,
                                    tag="s", name="s_sb")
                    nc.vector.tensor_mul(s4, ps_s, mask4)
                    eng_load["v"] += HL * CH / 0.96 + 150.0
                    s_sb = [s4[:, hl, :] for hl in range(HL)]

                    if k % 2 == 0:
                        po_pair = psout.tile([128, 2, N_IN], F32, tag="po",
                                             name="po")
                    po = po_pair[:, k % 2, :]
                    # 'start' zeroes the whole pair bank: only the first
                    # matmul of the EVEN chunk sets it.
                    bank_start = (k % 2 == 0)
                    # inter-chunk apply: po(t,i) += A^T P  (8 matmuls)
                    if k >= 1:
                        for hl in range(HL):
                            for jb in range(2):
                                nc.tensor.matmul(
                                    po, A4_sb[jb][:, hl, t0:t0 + CH],
                                    P_sb[hl // 2][jb][:, hl % 2, :],
                                    start=(bank_start and hl == 0
                                           and jb == 0),
                                    stop=False, skip_group_check=True)
                    # state update: P[p2][jb] += rN^T V, both heads per
                    # matmul; jb-outer so one rN stationary serves 2 matmuls
                    if k < NCH - 1:
                        for jb in range(2):
                            for p2 in range(2):
                                nc.tensor.matmul(
                                    P_ps[p2][jb],
                                    rN_sb[:, k, 128 * jb:128 * (jb + 1)],
                                    vt[p2],
                                    start=(k == 0), stop=(k == NCH - 2),
                                    skip_group_check=True)
                    # intra-chunk: po(t,i) += s^T V; with FP8_INTRA one
                    # DoubleRow matmul per head-pair sums both heads
                    if FP8_INTRA:
                        for p2 in range(2):
                            nc.tensor.matmul(
                                po, s4[:, 2 * p2:2 * p2 + 2, :], vt8[p2],
                                perf_mode=mybir.MatmulPerfMode.DoubleRow,
                                start=(k == 0 and p2 == 0),
                                stop=(k % 2 == 1 and p2 == 1),
                                skip_group_check=True)
                    else:
                        for hl in range(HL):
                            nc.tensor.matmul(
                                po, s_sb[hl], vt[hl // 2][:, hl % 2, :],
                                start=(k == 0 and hl == 0),
                                stop=(k % 2 == 1 and hl == HL - 1),
                                skip_group_check=True)
                    # drain the pair's output every other chunk
                    if k % 2 == 1:
                        ot = opool.tile([128, 2, N_IN], F32, tag="ot",
                                        name="ot")
                        copy_psum(ot, po_pair, 2 * N_IN)
                        for s in range(2):
                            nc.sync.dma_start(
                                out=out_d[t0 - CH + s * CH:
                                          t0 + s * CH, :],
                                in_=ot[:, s, :])

            if repeat == 1:
                body()
            elif repeat < 0:  # unrolled repeat (timing experiments)
                for _ in range(-repeat):
                    body()
            else:
                with tc.For_i(0, repeat, 1):
                    body()
    nc.compile()
    return nc


def _prep_in_maps(r_prime, E, Q, bf16=False):
    if bf16 == "fp16":
        cast_dt = np.float16
    elif bf16:
        import ml_dtypes
        cast_dt = ml_dtypes.bfloat16
    else:
        cast_dt = np.float32
    mask = _tri_mask()
    in_maps = []
    for c in range(N_CORES):
        b, hg = divmod(c, 2)
        heads = slice(4 * hg, 4 * hg + 4)
        rb = r_prime[0, b]                       # [T, I]
        rT = np.ascontiguousarray(rb.T).reshape(2, 128, N_T)
        rN = np.ascontiguousarray(
            rb.reshape(NCH, 128, N_IN).transpose(1, 0, 2))  # [u%128, k, j]
        Q4 = np.ascontiguousarray(Q[0, heads]).reshape(HL, 2, 128, N_IN)
        ET4 = np.ascontiguousarray(
            E[0, heads].transpose(0, 2, 1)).reshape(HL, 2, 128, N_IN)
        in_maps.append({"rT": rT.astype(cast_dt),
                        "rN": rN.astype(cast_dt),
                        "Q4": Q4.astype(cast_dt),
                        "ET4": ET4.astype(cast_dt),
                        "mask": mask.astype(cast_dt)})
    return in_maps


DTYPE = "fp16"  # float16 matmuls: full PE rate + fast weight loads


def kernel(r_prime, E, Q):
    from concourse import bass_utils

    if "nc" not in _cache:
        _cache["nc"] = _build_nc(bf16=DTYPE)
    nc = _cache["nc"]
    in_maps = _prep_in_maps(r_prime, E, Q, bf16=DTYPE)
    res = bass_utils.run_bass_kernel_spmd(nc, in_maps,
                                          core_ids=list(range(N_CORES)))
    out = np.zeros((1, 4, N_T, N_IN), dtype=np.float32)
    for b in range(4):
        out[0, b] = res.results[2 * b]["out"] + res.results[2 * b + 1]["out"]
    return out
